# revision 24
# baseline (speedup 1.0000x reference)
"""Trainium2 Bass/Tile kernel for nn_CAVAModule (cross-attention A/V alignment).

Math notes (exact simplifications of the reference):
  - delta = 2 + 4*sigmoid(clip(theta,-12,12)) is in [2, 6], so the mask
    center min(max(t+delta,0),t) == t for every t: the displacement-aware
    causal mask is a fixed 6-tap causal moving average, independent of theta.
  - The soft temporal shift composed with that moving average is a banded
    Toeplitz operator: three 128x128 blocks C00/Csub/Cdiag applied as PE
    matmuls against 128-token LayerNorm'd audio tiles.
  - LN means ride the projection matmul for free: the relaid weights carry
    an appended rowmean column, so psum col 256 is mean(x@W). The center
    is computed NEGATED ((m-y), one tensor_scalar with two scalar slots)
    so the Newton rsqrt can keep its natural negative root; variance is a
    square-accumulate on the centered values.
  - rsqrt via a 5-op magic-constant Newton chain; the x16 LN scale is
    folded into the magic constant (exponent +4), eps is dropped
    (var ~ 1 >> eps for random projections).
  - l2_normalize(LN(x)) collapses: vn = (x-m)*rstd / sqrt(DM) exactly.
  - The +-12 logit clip is dropped: sigmoid(+-12) lies far outside the
    [0.05, 0.95] gate clip, so the gate clip subsumes it.
  - Gate MLP in fp8e4 DoubleRow (2 k-tiles per matmul); W1 host-scaled
    (x32/x32/x2 => logits x512, gelu applies 1/512) and stored k-pair-
    interleaved so the moving operand streams contiguous byte pairs.

Structure: 4 groups of 4 token tiles, software-pipelined so phase C
(gate MLP + fuse) of group g-1 interleaves with phases A and B of group
g, keeping the PE fed and the pipeline head/tail short. Scalar-free
elementwise ops run on tile PAIRS ([P,512]) to amortize per-op engine
overhead; the gate tanh/clip chain is also paired.

Dtypes: video/audio/Wv/Wa bf16, context matmul f32r, gate path fp8/bf16,
output staged bf16 (host converts to f32).

Sharding: data-parallel over batch, one sample per NeuronCore, no
cross-core communication.
"""

import sys

for _p in ("/opt/trn_rl_repo",):
    if _p not in sys.path:
        sys.path.insert(0, _p)

import ml_dtypes
import numpy as np

import concourse.bacc as bacc
import concourse.bass as bass
import concourse.tile as tile
from concourse import mybir
from concourse.bass_utils import run_bass_kernel_spmd

F32 = mybir.dt.float32
F32R = mybir.dt.float32r
BF16 = mybir.dt.bfloat16
F8 = mybir.dt.float8e4
U32 = mybir.dt.uint32
ALU = mybir.AluOpType
ACT = mybir.ActivationFunctionType
DR = mybir.MatmulPerfMode.DoubleRow

B, T, VDIM, ADIM, DM, HID = 8, 2048, 1024, 768, 256, 1024
P = 128
NT = T // P          # 16 token tiles
KV = VDIM // P       # 8
KA = ADIM // P       # 6
KX = (3 * DM) // P   # 6
NB = 4               # token tiles per pipeline group
NG = NT // NB        # 4 groups
DMA = DM + 1         # projection free dim incl the mean column
LN_EPS = 1e-5        # dropped in-kernel: var >> eps for randn projections
WIN = 6              # mask window taps (tau in [t-5, t])
CGATE = 512.0        # gate-logit scale carried through the fp8 MLP
MAGIC = 0x5F3759DF   # rsqrt Newton seed
MAGIC16 = MAGIC + (4 << 23)   # seeds 16/sqrt(x) directly (exponent +4)

_nc_cache: dict = {}
_DEBUG_SIM = False   # set True in debug scripts: dump intermediates and
                     # replace Gelu with Copy (sim lacks Gelu)


def _build_cmats(delta: float) -> np.ndarray:
    """Three [tau, t] blocks of the combined shift+mask operator."""
    dl = min(max(delta, 0.0), float(T - 1))
    n = int(np.floor(dl))
    alpha = dl - n

    def row_w(t):
        w = np.zeros(2 * P, np.float64)
        m = min(t + 1, WIN)
        for s in range(max(0, t - (WIN - 1)), t + 1):
            i0 = min(max(s - n, 0), T - 1)
            i1 = min(i0 + 1, T - 1)
            w[i0] += (1.0 - alpha) / m
            w[i1] += alpha / m
        return w

    c00 = np.zeros((P, P), np.float64)
    csub = np.zeros((P, P), np.float64)
    cdiag = np.zeros((P, P), np.float64)
    for t in range(P):
        w = row_w(t)
        c00[:, t] = w[:P]
        w = row_w(P + t)
        csub[:, t] = w[:P]
        cdiag[:, t] = w[P:2 * P]
    return np.ascontiguousarray(np.stack([c00, csub, cdiag]).astype(np.float32))


def _build(bv_nz: bool, ba_nz: bool, b1_nz: bool, b2f: float):
    from contextlib import ExitStack

    nc = bacc.Bacc("TRN2", target_bir_lowering=False, debug=False, num_devices=8)

    vtc = nc.dram_tensor("vtc", [P, NT, KV, P], BF16, kind="ExternalInput")
    atc = nc.dram_tensor("atc", [P, NT, KA, P], BF16, kind="ExternalInput")
    wv = nc.dram_tensor("wv", [P, KV, DMA], BF16, kind="ExternalInput")
    wa = nc.dram_tensor("wa", [P, KA, DMA], BF16, kind="ExternalInput")
    w1 = nc.dram_tensor("w1", [P, KX // 2, HID, 2], F8, kind="ExternalInput")
    w2h = nc.dram_tensor("w2h", [HID], BF16, kind="ExternalInput")
    cm = nc.dram_tensor("cm", [P, 3, P], F32R, kind="ExternalInput")
    ident = nc.dram_tensor("ident", [P, P], BF16, kind="ExternalInput")
    if bv_nz:
        bvr = nc.dram_tensor("bvr", [1, DMA], F32R, kind="ExternalInput")
    if ba_nz:
        bar = nc.dram_tensor("bar", [1, DMA], F32R, kind="ExternalInput")
    if b1_nz:
        b1r = nc.dram_tensor("b1r", [1, HID], F32R, kind="ExternalInput")
    out = nc.dram_tensor("out", [P, NT, DM], BF16, kind="ExternalOutput")
    if _DEBUG_SIM:
        d_vln = nc.dram_tensor("d_vln", [P, NT, DM], BF16, kind="ExternalOutput")
        d_aln = nc.dram_tensor("d_aln", [P, NT, DM], F32R, kind="ExternalOutput")
        d_actx = nc.dram_tensor("d_actx", [P, NT, DM], BF16, kind="ExternalOutput")
        d_gg = nc.dram_tensor("d_gg", [P, NT], F32, kind="ExternalOutput")

    def bcast(handle_ap, n):
        return bass.AP(
            tensor=handle_ap.tensor, offset=handle_ap.offset, ap=[[0, P], [1, n]]
        )

    with tile.TileContext(nc) as tc:
        with ExitStack() as stk:
            singles = stk.enter_context(tc.tile_pool(name="singles", bufs=1))
            vchunk = stk.enter_context(tc.tile_pool(name="vchunk", bufs=2))
            achunk = stk.enter_context(tc.tile_pool(name="achunk", bufs=2))
            # PSUM banks (8): mm 3 + pc 1 + tr 2 + h 2
            wvc_pool = stk.enter_context(tc.tile_pool(name="wvc", bufs=NB + 2))
            wac_pool = stk.enter_context(tc.tile_pool(name="wac", bufs=NB + 1))
            vln_pool = stk.enter_context(tc.tile_pool(name="vlnp", bufs=NB // 2 + 2))
            ans_pool = stk.enter_context(tc.tile_pool(name="ansp", bufs=NB // 2 + 2))
            a_pool = stk.enter_context(tc.tile_pool(name="ap", bufs=3))
            cx_pool = stk.enter_context(tc.tile_pool(name="cxp", bufs=NB // 2 + 2))
            nv_pool = stk.enter_context(tc.tile_pool(name="nvp", bufs=4))
            xt_pool = stk.enter_context(tc.tile_pool(name="xtp", bufs=4))
            hbuf = stk.enter_context(tc.tile_pool(name="hb", bufs=3))
            obuf = stk.enter_context(tc.tile_pool(name="ob", bufs=2))
            small = stk.enter_context(tc.tile_pool(name="small", bufs=10))
            bsm = stk.enter_context(tc.tile_pool(name="bsm", bufs=2))
            psum_mm = stk.enter_context(
                tc.tile_pool(name="psum_mm", bufs=3, space="PSUM"))
            psum_pc = stk.enter_context(
                tc.tile_pool(name="psum_pc", bufs=1, space="PSUM"))
            psum_tr = stk.enter_context(
                tc.tile_pool(name="psum_tr", bufs=2, space="PSUM"))
            psum_h = stk.enter_context(
                tc.tile_pool(name="psum_h", bufs=2, space="PSUM"))
            # ---- persistent weights/constants ----
            # sync queue first: wv halves gate the very first matmuls
            wv1 = singles.tile([P, KV // 2, DMA], BF16)
            nc.sync.dma_start(out=wv1, in_=wv.ap()[:, 0:KV // 2, :])
            wv2 = singles.tile([P, KV - KV // 2, DMA], BF16)
            nc.sync.dma_start(out=wv2, in_=wv.ap()[:, KV // 2:KV, :])

            def wv_k(k):
                return wv1[:, k, :] if k < KV // 2 else wv2[:, k - KV // 2, :]

            # gpsimd queue: everything else, in need order
            wa_sb = singles.tile([P, KA, DMA], BF16)
            nc.gpsimd.dma_start(out=wa_sb, in_=wa.ap())
            cm_sb = singles.tile([P, 3, P], F32R)
            nc.gpsimd.dma_start(out=cm_sb, in_=cm.ap())
            id_sb = singles.tile([P, P], BF16)
            nc.gpsimd.dma_start(out=id_sb, in_=ident.ap())
            w1_sb = singles.tile([P, KX // 2, HID, 2], F8)
            nc.gpsimd.dma_start(out=w1_sb, in_=w1.ap())
            w2_sb = singles.tile([P, HID], BF16)
            nc.gpsimd.dma_start(out=w2_sb, in_=bcast(w2h.ap(), HID))
            kmag = singles.tile([P, 2, NB], U32)
            nc.vector.memset(kmag, MAGIC16)
            kmag1 = singles.tile([P, NB], U32)
            nc.vector.memset(kmag1, MAGIC)
            if bv_nz or ba_nz or b1_nz:
                ones_sb = singles.tile([1, P], F32R)
                nc.vector.memset(ones_sb, 1.0)
            if bv_nz:
                bv_sb = singles.tile([1, DMA], F32R)
                nc.gpsimd.dma_start(out=bv_sb, in_=bvr.ap())
            if ba_nz:
                ba_sb = singles.tile([1, DMA], F32R)
                nc.gpsimd.dma_start(out=ba_sb, in_=bar.ap())
            if b1_nz:
                b1_sb = singles.tile([1, HID], F32R)
                nc.gpsimd.dma_start(out=b1_sb, in_=b1r.ap())

            a_prev = [None]

            def phase_a_tile(g, j, vt_sb, at_sb, S):
                """Projections (+mean col), negated centering, ssq."""
                pv = psum_mm.tile([P, DMA], F32, tag="mm", name=f"pv{g}_{j}")
                for k in range(KV):
                    nc.tensor.matmul(pv, lhsT=vt_sb[:, j, k, :], rhs=wv_k(k),
                                     start=(k == 0),
                                     stop=(k == KV - 1 and not bv_nz))
                if bv_nz:
                    nc.tensor.matmul(pv, lhsT=ones_sb, rhs=bv_sb,
                                     start=False, stop=True)
                # w_vc = (mean - pv): negation cancels against the negative
                # Newton root in the LN scale
                w_vc = wvc_pool.tile([P, DM], BF16, tag="wvc")
                nc.vector.tensor_scalar(out=w_vc, in0=pv[:, 0:DM],
                                        scalar1=pv[:, DM:DMA], scalar2=-1.0,
                                        op0=ALU.subtract, op1=ALU.mult)
                S["w_vc"][j] = w_vc
                sqv = nv_pool.tile([P, DM], BF16, tag="sq")
                nc.vector.scalar_tensor_tensor(
                    out=sqv, in0=w_vc, scalar=0.0, in1=w_vc,
                    op0=ALU.bypass, op1=ALU.mult,
                    accum_out=S["ssqc"][:, 0, j:j + 1])

                pa = psum_mm.tile([P, DMA], F32, tag="mm", name=f"pa{g}_{j}")
                for k in range(KA):
                    nc.tensor.matmul(pa, lhsT=at_sb[:, j, k, :],
                                     rhs=wa_sb[:, k, :],
                                     start=(k == 0),
                                     stop=(k == KA - 1 and not ba_nz))
                if ba_nz:
                    nc.tensor.matmul(pa, lhsT=ones_sb, rhs=ba_sb,
                                     start=False, stop=True)
                w_ac = wac_pool.tile([P, DM], BF16, tag="wac")
                nc.vector.tensor_scalar(out=w_ac, in0=pa[:, 0:DM],
                                        scalar1=pa[:, DM:DMA], scalar2=-1.0,
                                        op0=ALU.subtract, op1=ALU.mult)
                S["w_ac"][j] = w_ac
                sqa = nv_pool.tile([P, DM], BF16, tag="sq")
                nc.vector.scalar_tensor_tensor(
                    out=sqa, in0=w_ac, scalar=0.0, in1=w_ac,
                    op0=ALU.bypass, op1=ALU.mult,
                    accum_out=S["ssqc"][:, 1, j:j + 1])

            def newton_rstd(g, S):
                """snr = -16/sqrt(ssqc) = -rstd_LN (negative root; cancels
                the negated center), whole group in a 5-op chain."""
                sh = bsm.tile([P, 2, NB], F32, tag="nsh", name="nsh", bufs=2)
                yb = bsm.tile([P, 2, NB], F32, tag="nyb", name="nyb", bufs=2)
                t0 = bsm.tile([P, 2, NB], F32, tag="nt0", name="nt0", bufs=2)
                nc.vector.tensor_scalar(out=sh.bitcast(U32),
                                        in0=S["ssqc"].bitcast(U32),
                                        scalar1=1, scalar2=None,
                                        op0=ALU.logical_shift_right)
                # yb = 16/sqrt(ssqc) seed (MAGIC16 folds the x16)
                nc.vector.tensor_tensor(out=yb.bitcast(U32), in0=kmag,
                                        in1=sh.bitcast(U32), op=ALU.subtract)
                nc.vector.tensor_tensor(out=t0, in0=yb, in1=yb, op=ALU.mult)
                nc.vector.scalar_tensor_tensor(out=t0, in0=S["ssqc"],
                                               scalar=1.0 / 512.0, in1=t0,
                                               op0=ALU.mult, op1=ALU.mult)
                nc.vector.scalar_tensor_tensor(out=S["snr"], in0=t0,
                                               scalar=1.5, in1=yb,
                                               op0=ALU.subtract, op1=ALU.mult)

            def phase_b_tile(g, j, S):
                """LN scales (ACT), context matmul, paired a_ctx copy + ssq."""
                if j % 2 == 0:
                    S["vlp"][j // 2] = vln_pool.tile([P, 2, DM], BF16,
                                                     tag="vln", name="vlnpair")
                vln = S["vlp"][j // 2][:, j % 2, :]
                nc.scalar.activation(out=vln, in_=S["w_vc"][j], func=ACT.Copy,
                                     scale=S["snr"][:, 0, j:j + 1])
                S["vln"][j] = vln
                a_sb = a_pool.tile([P, DM], F32R, tag="asb")
                nc.scalar.activation(out=a_sb, in_=S["w_ac"][j], func=ACT.Copy,
                                     scale=S["snr"][:, 1, j:j + 1])
                if j % 2 == 0:
                    S["pcpair"] = psum_pc.tile([P, 2, DM], F32, tag="pc",
                                               name="pcpair")
                pc = S["pcpair"][:, j % 2, :]
                if g == 0 and j == 0:
                    nc.tensor.matmul(pc, lhsT=cm_sb[:, 0, :], rhs=a_sb,
                                     start=True, stop=True)
                else:
                    nc.tensor.matmul(pc, lhsT=cm_sb[:, 1, :], rhs=a_prev[0],
                                     start=True, stop=False)
                    nc.tensor.matmul(pc, lhsT=cm_sb[:, 2, :], rhs=a_sb,
                                     start=False, stop=True)
                a_prev[0] = a_sb
                if _DEBUG_SIM:
                    nc.sync.dma_start(out=d_vln.ap()[:, g * NB + j, :], in_=vln)
                    nc.sync.dma_start(out=d_aln.ap()[:, g * NB + j, :], in_=a_sb)
                if j % 2 == 1:
                    cxp = cx_pool.tile([P, 2, DM], BF16, tag="cx")
                    nc.scalar.activation(out=cxp, in_=S["pcpair"],
                                         func=ACT.Copy)
                    S["actx"][j - 1] = cxp[:, 0, :]
                    S["actx"][j] = cxp[:, 1, :]
                    S["cxp"][j // 2] = cxp
                    if _DEBUG_SIM:
                        nc.sync.dma_start(
                            out=d_actx.ap()[:, g * NB + j - 1:g * NB + j + 1, :],
                            in_=cxp)
                    for jj in (j - 1, j):
                        sqd = nv_pool.tile([P, DM], BF16, tag="sq")
                        nc.vector.scalar_tensor_tensor(
                            out=sqd, in0=S["actx"][jj], scalar=1.0 / 256.0,
                            in1=S["actx"][jj], op0=ALU.mult, op1=ALU.mult,
                            accum_out=S["ssq"][:, jj:jj + 1])

            def newton_rn16(g, S):
                """rn16 = -16/||a_ctx|| (negative root, sign folded into W1),
                whole group, then the an_s = rn16*a_ctx pair tiles on ACT."""
                sh = bsm.tile([P, NB], F32, tag="msh", name="msh", bufs=2)
                yb = bsm.tile([P, NB], F32, tag="myb", name="myb", bufs=2)
                t0 = bsm.tile([P, NB], F32, tag="mt0", name="mt0", bufs=2)
                nc.vector.tensor_scalar(out=sh.bitcast(U32),
                                        in0=S["ssq"].bitcast(U32),
                                        scalar1=1, scalar2=None,
                                        op0=ALU.logical_shift_right)
                nc.vector.tensor_tensor(out=yb.bitcast(U32), in0=kmag1,
                                        in1=sh.bitcast(U32), op=ALU.subtract)
                nc.vector.tensor_tensor(out=t0, in0=yb, in1=yb, op=ALU.mult)
                nc.vector.scalar_tensor_tensor(out=t0, in0=S["ssq"],
                                               scalar=0.5, in1=t0,
                                               op0=ALU.mult, op1=ALU.mult)
                nc.vector.scalar_tensor_tensor(out=S["rn16"], in0=t0,
                                               scalar=1.5, in1=yb,
                                               op0=ALU.subtract, op1=ALU.mult)
                for j in range(NB):
                    if j % 2 == 0:
                        S["anp"][j // 2] = ans_pool.tile([P, 2, DM], BF16,
                                                         tag="ans", name="ansp")
                    nc.scalar.activation(out=S["anp"][j // 2][:, j % 2, :],
                                         in_=S["actx"][j], func=ACT.Copy,
                                         scale=S["rn16"][:, j:j + 1])

            def phase_c_tile(g, j, S):
                """Gate MLP for tile (g, j); fuse/output at pair granularity."""
                vln = S["vln"][j]
                an_s = S["anp"][j // 2][:, j % 2, :]
                if j % 2 == 0:
                    S["avnpair"] = nv_pool.tile([P, 2, DM], BF16, tag="avn",
                                                name="avnpair", bufs=2)
                    nc.vector.tensor_tensor(out=S["avnpair"],
                                            in0=S["anp"][j // 2],
                                            in1=S["vlp"][j // 2], op=ALU.mult)
                avn = S["avnpair"][:, j % 2, :]
                pt = psum_tr.tile([P, KX, P], BF16, tag="pt")
                for k in range(2):
                    nc.tensor.transpose(pt[:, k, :],
                                        an_s[:, k * P:(k + 1) * P], id_sb)
                for k in range(2):
                    nc.tensor.transpose(pt[:, 2 + k, :],
                                        vln[:, k * P:(k + 1) * P], id_sb)
                for k in range(2):
                    nc.tensor.transpose(pt[:, 4 + k, :],
                                        avn[:, k * P:(k + 1) * P], id_sb)
                xt = xt_pool.tile([P, KX, P], F8, tag="xt")
                nc.scalar.activation(out=xt, in_=pt, func=ACT.Copy)

                ph0 = psum_h.tile([P, 512], F32, tag="h")
                ph1 = psum_h.tile([P, 512], F32, tag="h")
                hh = hbuf.tile([P, HID], BF16, tag="hh")
                for nh, psl in ((0, ph0), (1, ph1)):
                    for kk in range(3):
                        nc.tensor.matmul(
                            psl, lhsT=xt[:, 2 * kk:2 * kk + 2, :],
                            rhs=w1_sb[:, kk, nh * 512:(nh + 1) * 512, :]
                            .rearrange("p n i -> p i n"),
                            start=(kk == 0), stop=(kk == 2 and not b1_nz),
                            perf_mode=DR)
                    if b1_nz:
                        nc.tensor.matmul(psl, lhsT=ones_sb,
                                         rhs=b1_sb[:, nh * 512:(nh + 1) * 512],
                                         start=False, stop=True)
                    nc.scalar.activation(out=hh[:, nh * 512:(nh + 1) * 512],
                                         in_=psl,
                                         func=ACT.Copy if _DEBUG_SIM else ACT.Gelu,
                                         scale=1.0 / CGATE)

                if j % 2 == 0:
                    S["lgp"] = small.tile([P, 2], F32, tag="lgp", name="lgp")
                lscd = hbuf.tile([P, HID], BF16, tag="lsc")
                nc.vector.scalar_tensor_tensor(
                    out=lscd, in0=hh, scalar=0.0, in1=w2_sb, op0=ALU.bypass,
                    op1=ALU.mult, accum_out=S["lgp"][:, j % 2:j % 2 + 1])
                if j % 2 == 0:
                    return

                # ---- pair tail: gate chain + fuse + output DMA ----
                i_glob = g * NB + j
                # +-12 logit clip dropped: subsumed by the [0.05,0.95] clip
                ggp = small.tile([P, 2], F32, tag="ggp", name="ggp")
                nc.scalar.activation(out=ggp, in_=S["lgp"], func=ACT.Tanh,
                                     scale=0.5, bias=0.5 * b2f)
                nc.vector.tensor_scalar(out=ggp, in0=ggp, scalar1=0.5,
                                        scalar2=0.5, op0=ALU.mult, op1=ALU.add)
                nc.vector.tensor_scalar(out=ggp, in0=ggp, scalar1=0.05,
                                        scalar2=0.95, op0=ALU.max, op1=ALU.min)
                if _DEBUG_SIM:
                    nc.sync.dma_start(
                        out=d_gg.ap()[:, i_glob - 1:i_glob + 1], in_=ggp)

                # fused = g*a_ctx + (1-g)*vln = vln + g*(a_ctx - vln)
                ddp = nv_pool.tile([P, 2, DM], BF16, tag="dd", bufs=2)
                nc.vector.tensor_tensor(out=ddp, in0=S["cxp"][j // 2],
                                        in1=S["vlp"][j // 2], op=ALU.subtract)
                ob = obuf.tile([P, 2, DM], BF16, tag="ob", name="ob")
                for jj in (0, 1):
                    nc.vector.scalar_tensor_tensor(
                        out=ob[:, jj, :], in0=ddp[:, jj, :],
                        scalar=ggp[:, jj:jj + 1],
                        in1=S["vlp"][j // 2][:, jj, :],
                        op0=ALU.mult, op1=ALU.add)
                nc.sync.dma_start(
                    out=out.ap()[:, i_glob - 1:i_glob + 1, :], in_=ob)

            def new_state(g):
                return {
                    "ssqc": bsm.tile([P, 2, NB], F32, tag="ssqc", name="ssqc"),
                    "snr": bsm.tile([P, 2, NB], F32, tag="snr", name="snr"),
                    "ssq": bsm.tile([P, NB], F32, tag="ssq", name="ssq"),
                    "rn16": bsm.tile([P, NB], F32, tag="rn16", name="rn16"),
                    "w_vc": [None] * NB, "w_ac": [None] * NB,
                    "vln": [None] * NB, "actx": [None] * NB,
                    "cxp": [None] * (NB // 2), "vlp": [None] * (NB // 2),
                    "anp": [None] * (NB // 2),
                    "avnpair": None, "pcpair": None, "lgp": None,
                }

            Sprev = None
            h = NB // 2
            for g in range(NG):
                i0 = g * NB
                vt_sb = vchunk.tile([P, NB, KV, P], BF16, tag="vt")
                at_sb = achunk.tile([P, NB, KA, P], BF16, tag="at")
                if g == 0:
                    # per-tile DMAs so tile j's matmuls start as soon as its
                    # own slice lands
                    for j in range(NB):
                        nc.sync.dma_start(
                            out=vt_sb[:, j, :, :], in_=vtc.ap()[:, j, :, :])
                        nc.sync.dma_start(
                            out=at_sb[:, j, :, :], in_=atc.ap()[:, j, :, :])
                else:
                    nc.sync.dma_start(
                        out=vt_sb, in_=vtc.ap()[:, i0:i0 + NB, :, :])
                    nc.sync.dma_start(
                        out=at_sb, in_=atc.ap()[:, i0:i0 + NB, :, :])

                S = new_state(g)
                # phase C of group g-1 split across phases A and B of g:
                # 2 tiles into the A-interleave (A is PE-heavy already),
                # 2 tiles into the B-interleave (B is PE-light)
                for j in range(NB):
                    phase_a_tile(g, j, vt_sb, at_sb, S)
                    if Sprev is not None and j % 2 == 1:
                        phase_c_tile(g - 1, j - 1, Sprev)
                        phase_c_tile(g - 1, j, Sprev)
                newton_rstd(g, S)
                for j in range(NB):
                    phase_b_tile(g, j, S)
                newton_rn16(g, S)
                if g == NG - 1:
                    for j in range(NB):
                        phase_c_tile(g, j, S)
                Sprev = S

    nc.compile()
    return nc


def _prepare_in_maps(video_seq, audio_seq, Wv, bv, Wa, ba, theta, W1, b1, W2, b2):
    bf16 = ml_dtypes.bfloat16
    f8 = ml_dtypes.float8_e4m3
    video_seq = np.asarray(video_seq, np.float32)
    audio_seq = np.asarray(audio_seq, np.float32)
    th = float(np.clip(np.float32(theta), -12.0, 12.0))
    delta = 2.0 + 4.0 / (1.0 + np.exp(-th))
    cmats = _build_cmats(float(delta))

    bv_nz = bool(np.any(np.asarray(bv) != 0))
    ba_nz = bool(np.any(np.asarray(ba) != 0))
    b1_nz = bool(np.any(np.asarray(b1) != 0))
    b2f = float(np.asarray(b2).reshape(-1)[0])

    W1f = np.asarray(W1, np.float32)
    W1s = np.empty_like(W1f)
    W1s[:DM] = W1f[:DM] * (-CGATE / 16.0)          # sign-folds the negative rn16
    W1s[DM:2 * DM] = W1f[DM:2 * DM] * (CGATE / 16.0)
    W1s[2 * DM:] = W1f[2 * DM:] * (-CGATE / 256.0)
    # k-pair interleaved: [P, KX//2, HID, 2]
    w1r = np.ascontiguousarray(
        W1s.astype(f8).reshape(KX // 2, 2, P, HID).transpose(2, 0, 3, 1))

    def relay_aug(w, ko):
        # [K, DM] -> [P, ko, DM+1] with col DM = rowmean (the LN mean rides
        # the projection matmul for free)
        w = np.asarray(w, np.float32)
        waug = np.concatenate([w, w.mean(axis=1, keepdims=True)], axis=1)
        n = waug.shape[1]
        return np.ascontiguousarray(
            waug.astype(bf16).reshape(ko, P, n).transpose(1, 0, 2))

    shared = {
        "wv": relay_aug(Wv, KV),
        "wa": relay_aug(Wa, KA),
        "w1": w1r,
        "w2h": np.ascontiguousarray(
            np.asarray(W2, np.float32).reshape(HID).astype(bf16)),
        "cm": np.ascontiguousarray(cmats.transpose(1, 0, 2)),
        "ident": np.eye(P, dtype=np.float32).astype(bf16),
    }
    if bv_nz:
        bvf = np.asarray(bv, np.float32).reshape(1, DM)
        shared["bvr"] = np.ascontiguousarray(
            np.concatenate([bvf, bvf.mean(axis=1, keepdims=True)], axis=1))
    if ba_nz:
        baf = np.asarray(ba, np.float32).reshape(1, DM)
        shared["bar"] = np.ascontiguousarray(
            np.concatenate([baf, baf.mean(axis=1, keepdims=True)], axis=1))
    if b1_nz:
        shared["b1r"] = np.ascontiguousarray(
            np.asarray(b1, np.float32).reshape(1, HID) * CGATE)

    in_maps = []
    for b in range(B):
        m = dict(shared)
        m["vtc"] = np.ascontiguousarray(
            video_seq[b].T.astype(bf16).reshape(KV, P, NT, P).transpose(1, 2, 0, 3))
        m["atc"] = np.ascontiguousarray(
            audio_seq[b].T.astype(bf16).reshape(KA, P, NT, P).transpose(1, 2, 0, 3))
        in_maps.append(m)
    return in_maps, (bv_nz, ba_nz, b1_nz, b2f)


def kernel(video_seq, audio_seq, Wv, bv, Wa, ba, theta, W1, b1, W2, b2):
    in_maps, key = _prepare_in_maps(video_seq, audio_seq, Wv, bv, Wa, ba,
                                    theta, W1, b1, W2, b2)
    if key not in _nc_cache:
        _nc_cache[key] = _build(*key)
    nc = _nc_cache[key]
    res = run_bass_kernel_spmd(nc, in_maps, list(range(B)))
    outs = []
    for i in range(B):
        r = np.asarray(res.results[i]["out"]).astype(np.float32)
        outs.append(np.ascontiguousarray(
            r.reshape(P, NT, DM).transpose(1, 0, 2).reshape(T, DM)))
    return np.stack(outs)
